# revision 5
# baseline (speedup 1.0000x reference)
"""TRN2 Bass kernel v2 for nn_DecoderLayer_70781061038465 (Falcon-7B style
decoder layer: fractured LayerNorm -> parallel MQA attention + MLP -> residual).

Sharding: 8-way tensor parallelism, no collectives. Each core computes a
partial sum of (attn_out + mlp_out) over its head/MLP shard; the host reduces
the 8 partials and adds the residual.

Design (vs the naive structure):
  - LayerNorm folded entirely into host prep: x-tilde = rstd*(x-mu) prescaled,
    ones/mr columns appended, and the whole operand pre-TRANSPOSED host-side so
    the device only streams contiguous feature-major row-blocks (no XBAR).
  - fp8-e4m3 DoubleRow matmuls with 3-term residual correction for the big
    GEMMs: W ~ w8+dw, x ~ x8+r (both packed as interleaved fp8 pairs); one
    DR matmul per k-tile computes w8^T r + dw^T x8, k-pair DR matmuls compute
    the main w8^T x8 — 0.75x the bf16 PE cycles at BETTER-than-bf16 accuracy.
    Weights carry a x64 prescale (undone on eviction) to keep dw normal-range.
  - Attention (bf16) pipelined inside the MLP projection: per head
    [scores 6][proj hb0][scores 6][proj hb1][ctx], so Act-engine exp latency
    and PSUM-bank recycling hide under matmuls. Gelu is deferred to batched
    in-place groups to avoid Exp<->Gelu activation-table thrash, fused with
    the fp8 (gr|g8) pair quantization feeding the down-projection.
  - dense+down after attention: ct part bf16, gt part fp8-DR, one fused PSUM
    accumulation per 512-wide output chunk across all 8 token row-blocks.
  - Batch pipeline: batch 1's x loads stream during batch 0's dense phase;
    weight tiles prefetched across all phase borders; startup runs the first
    two m-tiles at 128-token-chunk granularity so the PE starts ~11us in.
  - q heads packed two-per-partition-line ([128, 5, S]) via host-side head
    interleave (m, m+5); k duplicated to both partition halves by the PE.
"""
import sys
if "/opt/trn_rl_repo" not in sys.path:
    sys.path.insert(0, "/opt/trn_rl_repo")

from contextlib import ExitStack

import numpy as np
import ml_dtypes

import concourse.bass as bass
import concourse.tile as tile
from concourse import bacc, mybir
from concourse.bass_utils import run_bass_kernel_spmd

F32 = mybir.dt.float32
BF16 = mybir.dt.bfloat16
FP8 = mybir.dt.float8e4
DR = mybir.MatmulPerfMode.DoubleRow
AF = mybir.ActivationFunctionType
MUL = mybir.AluOpType.mult
ADD = mybir.AluOpType.add

# problem shapes (hardcoded per contract)
B, S, H, NH, HD = 2, 1024, 4544, 71, 64
T = B * S                 # 2048 tokens
HP = 4608                 # padded hidden (36*128)
HH = HP // 2              # 2304 half-row width
KT = HP // 128            # 36 contraction tiles
NHC = 10                  # heads per core
QC = NHC * HD             # 640 q channels/core
F4 = 4 * H                # 18176
F4C_REAL = F4 // 8        # 2272
F4C = 2304                # padded (18*128)
MT = 24                   # proj m-tiles: 5 q + 1 kv + 18 mlp
DDK = 23                  # dense contraction tiles: 5 ct + 18 gt
KKG = 6                   # dense k-groups of 4
FC = HP // 512            # 9 output f-chunks
EPS = 1e-5
NEG = -30.0
SW = 64.0                 # fp8 weight prescale (undone at psum eviction)

_CACHE = {}


def _et_chunk(skt, sqc):
    # triangular et storage: (skt<4, sqc in {0,1}) -> 0..7 ; (skt>=4, sqc=1) -> 8..11
    return 2 * skt + sqc if skt < 4 else 4 + skt


def _build():
    nc = bacc.Bacc("TRN2", target_bir_lowering=False, debug=False)
    # x-tilde pre-transposed host-side, fp8 (r|x8) pairs:
    #   [row-block, 128 features, ko, slot(0=r,1=x8), 128 tok]
    xb_d = nc.dram_tensor("xb", [16, 128, KT, 2, 128], FP8, kind="ExternalInput")
    # proj weights fp8 (w8|dw) pairs: [m, 128 row-in-tile, ko, slot, out-ch]
    wpk_d = nc.dram_tensor("wpk", [MT, 128, KT * 2 * 128], FP8,
                           kind="ExternalInput")
    # dense ct-part weights (bf16, x64-prescaled): [128 row-in-tile, 5, HP]
    wddc_d = nc.dram_tensor("wddc", [128, 5, HP], BF16, kind="ExternalInput")
    # dense gt-part weights fp8: rows 0..8 main (w8_2k|w8_2k+1),
    # rows 9..26 corr (w8_k|dw_k); [128, 27, 2, HP]
    wddg_d = nc.dram_tensor("wddg", [128, 27, 2, HP], FP8, kind="ExternalInput")
    cs_d = nc.dram_tensor("csn", [2, 128, S], BF16, kind="ExternalInput")
    dm_d = nc.dram_tensor("dmask", [128, 128], F32, kind="ExternalInput")
    out_d = nc.dram_tensor("out", [T, HP], F32, kind="ExternalOutput")

    xb = xb_d.ap()
    wpk = wpk_d.ap()
    wddc = wddc_d.ap()
    wddg = wddg_d.ap()
    out = out_d.ap()

    with tile.TileContext(nc) as tc, ExitStack() as ctx:
        def pool(name, bufs, space="SBUF"):
            return ctx.enter_context(tc.tile_pool(name=name, bufs=bufs, space=space))

        const = pool("const", 1)
        wpool = pool("wp", 2)
        gts = pool("gts", 4)      # pre-gelu bf16 staging
        res = pool("res", 1)      # per-batch residents: qt/kt2/vt/gt/ct + xt
        et_p = pool("et", 1)
        c2_p = pool("c2", 1)
        wdp = pool("wdp", 2)
        wcp = pool("wcp", 2)
        outp = pool("outp", 2)
        small = pool("small", 4)
        psp = pool("psp", 8, space="PSUM")

        cos_sb = const.tile([128, S], BF16, tag="cos")
        nc.sync.dma_start(cos_sb[:], cs_d.ap()[0])
        sin_sb = const.tile([128, S], BF16, tag="sin")
        nc.sync.dma_start(sin_sb[:], cs_d.ap()[1])
        dmaskT = const.tile([128, 128], F32, tag="dmaskT")
        nc.sync.dma_start(dmaskT[:], dm_d.ap())

        state = {}
        wt_cache = {}
        wdt_cache = {}

        def alloc_batch(b):
            state["qt"] = res.tile([128, 5, S], BF16, tag="qt", name=f"qt{b}")
            state["kt2"] = res.tile([128, S], BF16, tag="kt2", name=f"kt2{b}")
            state["vt"] = res.tile([128, 8, 72], BF16, tag="vt", name=f"vt{b}")
            # fp8 (gr|g8) pairs of the gelu output
            state["gt8"] = res.tile([128, 18, 2, S], FP8, tag="gt8",
                                    name=f"gt8{b}")
            # [p, sqt, head-pair, t]: mid-dim strides keep the transpose AP 3D
            state["ct"] = res.tile([128, 8, 5, 128], BF16, tag="ct", name=f"ct{b}")
            nc.vector.memset(state["vt"][:, :, 64:65], 1.0)

        def alloc_xt(b):
            # [p features, row-block, ko, slot(r|x8), tok]: row-block-major so
            # each host-transposed block loads as one contiguous 9216B DMA
            state["xt"] = res.tile([128, 8, KT, 2, 128], FP8, tag="xt",
                                   name=f"xt{b}")

        def load_wt(b, m):
            wt = wpool.tile([128, KT, 2, 128], FP8, tag="wt", name=f"wt{b}_{m}")
            nc.sync.dma_start(
                wt[:], wpk[m].rearrange("p (ko s c) -> p ko s c", s=2, c=128))
            wt_cache[(b, m)] = wt

        def load_wdt(b, fc, part):
            # part 0/1/2 -> wddg row-groups [0:9], [9:18], [18:27]
            fcols = slice(fc * 512, (fc + 1) * 512)
            wdt = wdp.tile([128, 9, 2, 512], FP8, tag="wdt",
                           name=f"wdt{b}_{fc}_{part}")
            nc.sync.dma_start(wdt[:], wddg[:, 9 * part:9 * part + 9, :, fcols])
            wdt_cache[(b, fc, part)] = wdt

        def load_wdc(b, fc):
            fcols = slice(fc * 512, (fc + 1) * 512)
            wdc = wcp.tile([128, 5, 512], BF16, tag="wdc", name=f"wdc{b}_{fc}")
            nc.sync.dma_start(wdc[:], wddc[:, :, fcols])
            wdt_cache[(b, fc, "c")] = wdc

        def phase_a(b, r):
            """Load one host-transposed, host-LN-prescaled fp8 row-block of
            (r|x8)-packed x-tilde^T straight into xt."""
            nc.sync.dma_start(state["xt"][:, r, :, :, :], xb[b * 8 + r])

        def proj_evict(b, m, hb, ps, wt):
            # psum carries the x64 weight prescale; undo it on eviction
            hcols = slice(hb * 512, hb * 512 + 512)
            if m < 5:
                nc.vector.tensor_scalar_mul(state["qt"][:, m, hcols], ps[:],
                                            1.0 / SW)
            elif m == 5:
                nc.vector.tensor_scalar_mul(state["kt2"][0:64, hcols],
                                            ps[0:64, :], 1.0 / SW)
                nc.vector.tensor_scalar_mul(state["kt2"][64:128, hcols],
                                            ps[0:64, :], 1.0 / SW)
                for j in range(4):
                    r2 = hb * 4 + j
                    pv = psp.tile([128, 72], F32, tag="ps", name=f"pv{b}_{r2}")
                    xr = state["xt"]
                    for kp in range(KT // 2):
                        nc.tensor.matmul(
                            pv[:, :64],
                            xr[:, r2, 2 * kp:2 * kp + 2, 1, :],
                            wt[:, 2 * kp:2 * kp + 2, 0, 64:128],
                            start=(kp == 0), stop=False, perf_mode=DR)
                    for k in range(KT):
                        nc.tensor.matmul(
                            pv[:, :64],
                            xr[:, r2, k, :, :],
                            wt[:, k, :, 64:128],
                            start=False, stop=(k == KT - 1), perf_mode=DR)
                    nc.vector.tensor_scalar_mul(state["vt"][:, r2, :64],
                                                pv[:, :64], 1.0 / SW)
            else:
                # raw (pre-gelu) bf16 evict to scratch; gelu + fp8 pair
                # quantization happen later in batches
                tt = m - 6
                if hb == 0:
                    state[f"scr{tt}"] = gts.tile([128, S], BF16, tag="scr",
                                                 name=f"scr{b}_{tt}")
                nc.vector.tensor_scalar_mul(state[f"scr{tt}"][:, hcols],
                                            ps[:], 1.0 / SW)

        def proj_hb(b, m, hb):
            xt = state["xt"]
            if (b, m) not in wt_cache:
                load_wt(b, m)
            wt = wt_cache[(b, m)]
            hcols = slice(hb * 512, hb * 512 + 512)
            ps = psp.tile([128, 512], F32, tag="ps", name=f"ps{b}_{m}_{hb}")
            rb = slice(hb * 4, (hb + 1) * 4)
            for kp in range(KT // 2):
                nc.tensor.matmul(
                    ps[:], wt[:, 2 * kp:2 * kp + 2, 0, :],
                    xt[:, rb, 2 * kp:2 * kp + 2, 1, :].rearrange(
                        "p r k t -> p k r t"),
                    start=(kp == 0), stop=False, perf_mode=DR)
            for k in range(KT):
                nc.tensor.matmul(
                    ps[:], wt[:, k, :, :],
                    xt[:, rb, k, :, :].rearrange("p r j t -> p j r t"),
                    start=False, stop=(k == KT - 1), perf_mode=DR)
            proj_evict(b, m, hb, ps, wt)
            if hb == 1:
                wt_cache.pop((b, m))

        def proj_m(b, m):
            proj_hb(b, m, 0)
            proj_hb(b, m, 1)

        chunk_ps = {}

        def proj_chunk(b, m, c):
            """128-token-chunk projection for startup: chunk c becomes ready
            as soon as row-block c is transposed."""
            xt = state["xt"]
            wt = wt_cache[(b, m)]
            hb = c // 4
            key = (b, m, hb)
            if key not in chunk_ps:
                chunk_ps[key] = psp.tile([128, 512], F32, tag="ps",
                                         name=f"ps{b}_{m}_{hb}")
            ps = chunk_ps[key]
            col = (c % 4) * 128
            for kp in range(KT // 2):
                nc.tensor.matmul(
                    ps[:, col:col + 128], wt[:, 2 * kp:2 * kp + 2, 0, :],
                    xt[:, c, 2 * kp:2 * kp + 2, 1, :],
                    start=(kp == 0), stop=False, perf_mode=DR)
            for k in range(KT):
                nc.tensor.matmul(
                    ps[:, col:col + 128], wt[:, k, :, :],
                    xt[:, c, k, :, :],
                    start=False, stop=(k == KT - 1), perf_mode=DR)
            if c % 4 == 3:
                proj_evict(b, m, hb, chunk_ps.pop(key), wt)
                if hb == 1:
                    wt_cache.pop((b, m))

        def gelu_batch(tiles):
            gt8 = state["gt8"]
            for tt in tiles:
                scr = state.pop(f"scr{tt}")
                nc.scalar.activation(scr[:], scr[:], AF.Gelu)
                nc.vector.tensor_copy(gt8[:, tt, 1, :], scr[:])
                nc.vector.tensor_tensor(gt8[:, tt, 0, :], scr[:],
                                        gt8[:, tt, 1, :],
                                        op=mybir.AluOpType.subtract)

        def rope_tile(t):
            # t: [128, S] bf16; rotate-half on both 64-row halves
            rot = c2_p.tile([128, 8, 128], BF16, tag="c2", name="rot")
            rot = rot[:].rearrange("p a b -> p (a b)")
            nc.vector.tensor_scalar_mul(rot[0:32, :], t[32:64, :], -1.0)
            nc.vector.tensor_copy(rot[32:64, :], t[0:32, :])
            nc.vector.tensor_scalar_mul(rot[64:96, :], t[96:128, :], -1.0)
            nc.vector.tensor_copy(rot[96:128, :], t[64:96, :])
            nc.vector.tensor_mul(t, t, cos_sb[:])
            nc.vector.tensor_mul(rot[:], rot[:], sin_sb[:])
            nc.vector.tensor_add(t, t, rot[:])

        def rope_all(b):
            for slot in range(5):
                rope_tile(state["qt"][:, slot, :])
            rope_tile(state["kt2"][:])

        def attn_scores(b, h, part):
            half, slot = h // 5, h % 5
            base = 64 * half
            qt, kt2 = state["qt"], state["kt2"]
            if part == 0:
                et = et_p.tile([128, 12, 512], BF16, tag="et",
                               name=f"et{b}_{h}")
                state["et"] = et
            et = state["et"]
            skts = range(0, 3) if part == 0 else range(3, 8)
            for skt in skts:
                for sqc in range(skt // 4, 2):
                    sp = psp.tile([128, 512], F32, tag="ps",
                                  name=f"sp{b}_{h}_{skt}_{sqc}")
                    nc.tensor.matmul(
                        sp[:], kt2[base:base + 64, skt * 128:(skt + 1) * 128],
                        qt[base:base + 64, slot, sqc * 512:(sqc + 1) * 512],
                        start=True, stop=True)
                    if skt // 4 == sqc:
                        lc = skt * 128 - sqc * 512
                        nc.vector.tensor_tensor(
                            sp[:, lc:lc + 128], sp[:, lc:lc + 128],
                            dmaskT[:], op=ADD)
                    nc.scalar.activation(
                        et[:, _et_chunk(skt, sqc), :], sp[:], AF.Exp)

        def attn_ctx(b, h):
            vt, ct, et = state["vt"], state["ct"], state["et"]
            if h % 2 == 0:
                state["c2"] = c2_p.tile([128, 8, 128], BF16, tag="c2",
                                        name=f"c2{b}_{h}")
            c2 = state["c2"]
            for sqt in range(8):
                cp = psp.tile([128, 72], F32, tag="ps", name=f"cp{b}_{h}_{sqt}")
                sqc = sqt // 4
                off = sqt * 128 - sqc * 512
                for skt in range(sqt + 1):
                    nc.tensor.matmul(
                        cp[:, :65],
                        et[:, _et_chunk(skt, sqc), off:off + 128],
                        vt[:, skt, :65],
                        start=(skt == 0), stop=(skt == sqt))
                recd = small.tile([128, 1], F32, tag="recd")
                nc.vector.reciprocal(recd[:], cp[:, 64:65])
                nc.vector.tensor_scalar_mul(
                    c2[:, sqt, (h % 2) * 64:(h % 2) * 64 + 64],
                    cp[:, :64], recd[:])
            if h % 2 == 1:
                nc.sync.dma_start_transpose(
                    ct[:, :, h // 2, :],
                    c2[:].rearrange("p a b -> p (a b)"))

        def dense_fc(b, fc, extra=None):
            gt8, ct = state["gt8"], state["ct"]
            fcols = slice(fc * 512, (fc + 1) * 512)
            pss = [psp.tile([128, 512], F32, tag="ps",
                            name=f"pd{b}_{fc}_{i}") for i in range(8)]
            # ct part, bf16
            if (b, fc, "c") not in wdt_cache:
                load_wdc(b, fc)
            wdc = wdt_cache.pop((b, fc, "c"))
            if (b, fc, 0) not in wdt_cache:
                load_wdt(b, fc, 0)
            for kk in range(5):
                for r in range(8):
                    nc.tensor.matmul(pss[r][:], ct[:, r, kk, :],
                                     wdc[:, kk, :],
                                     start=(kk == 0), stop=False)
            # gt main: g8 k-pairs x w8 k-pairs
            wdt = wdt_cache.pop((b, fc, 0))
            load_wdt(b, fc, 1)
            for kp in range(9):
                for r in range(8):
                    rcols = slice(r * 128, (r + 1) * 128)
                    nc.tensor.matmul(
                        pss[r][:],
                        gt8[:, 2 * kp:2 * kp + 2, 1, rcols],
                        wdt[:, kp, :, :],
                        start=False, stop=False, perf_mode=DR)
            # gt corrections: (gr|g8) x (w8|dw)
            for part in (1, 2):
                wdt = wdt_cache.pop((b, fc, part))
                if part == 1:
                    load_wdt(b, fc, 2)
                    if fc + 1 < FC:
                        load_wdc(b, fc + 1)
                    if extra is not None:
                        extra()
                for kx in range(9):
                    k = (part - 1) * 9 + kx
                    for r in range(8):
                        rcols = slice(r * 128, (r + 1) * 128)
                        nc.tensor.matmul(
                            pss[r][:],
                            gt8[:, k, :, rcols],
                            wdt[:, kx, :, :],
                            start=False, stop=(k == 17), perf_mode=DR)
            for r in range(8):
                osb = outp.tile([128, 512], F32, tag="osb")
                nc.vector.tensor_scalar_mul(osb[:], pss[r][:], 1.0 / SW)
                nc.sync.dma_start(
                    out[b * S + r * 128: b * S + (r + 1) * 128, fcols], osb[:])

        # batched gelu groups: after unit h, which gt tiles to activate
        GELU_SCHED = {2: range(0, 3), 4: range(3, 5), 6: range(5, 7),
                      8: range(7, 9), 9: range(9, 12)}

        def batch_body(b, startup=False, pipelined_next=False):
            alloc_batch(b)
            if startup:
                # chunked m0/m1: each 128-token chunk starts right after its
                # row-block transpose lands
                load_wt(b, 0)
                phase_a(b, 0)
                load_wt(b, 1)
                for r in range(8):
                    if r < 7:
                        phase_a(b, r + 1)
                    proj_chunk(b, 0, r)
                    proj_chunk(b, 1, r)
                for m in range(2, 6):
                    proj_m(b, m)
            else:
                for m in range(6):
                    proj_m(b, m)
            rope_all(b)
            proj_m(b, 6)
            proj_m(b, 7)
            for h in range(10):
                attn_scores(b, h, 0)
                proj_hb(b, 8 + h, 0)
                attn_scores(b, h, 1)
                proj_hb(b, 8 + h, 1)
                attn_ctx(b, h)
                if h in GELU_SCHED:
                    gelu_batch(GELU_SCHED[h])
            load_wdc(b, 0)
            load_wdt(b, 0, 0)
            for m in range(18, 20):
                proj_m(b, m)
            gelu_batch(range(12, 14))
            for m in range(20, 22):
                proj_m(b, m)
            gelu_batch(range(14, 16))
            for m in range(22, 24):
                proj_m(b, m)
            gelu_batch(range(16, 18))
            if pipelined_next:
                alloc_xt(1)

                def step(fc):
                    def run():
                        phase_a(1, fc - 1)
                        if fc == 6:
                            load_wt(1, 0)
                        elif fc == 7:
                            load_wt(1, 1)
                    return run
            for fc in range(FC):
                extra = step(fc) if (pipelined_next and 1 <= fc <= 8) else None
                dense_fc(b, fc, extra=extra)

        alloc_xt(0)
        batch_body(0, startup=True, pipelined_next=True)
        batch_body(1)

    nc.compile()
    return nc


def _prep_inputs(hidden_states, cos, sin, ln_w1, ln_b1, ln_w2, ln_b2,
                 wq, wk, wv, w_dense, w_h4h, w_4hh):
    f32 = np.float32
    bf = ml_dtypes.bfloat16
    e4m3 = ml_dtypes.float8_e4m3
    lnw = np.concatenate([np.asarray(ln_w1), np.asarray(ln_w2)]).astype(np.float64)
    lnb = np.concatenate([np.asarray(ln_b1), np.asarray(ln_b2)]).astype(np.float64)

    def pack(Wc, scale=1.0, prescale=1.0):
        # Wc [O, H] -> [HP, O] f64: ln-folded + bias row + colsum row + pad.
        # prescale multiplies all rows EXCEPT the colsum row (its x-side
        # partner, the mr column, carries the prescale instead).
        W64 = Wc.astype(np.float64) * scale
        Wp = W64 * lnw                      # [O, H]
        bias = W64 @ lnb                    # [O]
        cw = Wp.sum(axis=1)                 # [O]
        O = Wc.shape[0]
        outw = np.zeros((HP, O), np.float64)
        outw[:H] = Wp.T * prescale
        outw[H] = bias * prescale
        outw[H + 1] = cw
        return outw

    def fp8_pair(M):
        # M f64 [HP, O] -> (w8, dw) e4m3
        w8 = M.astype(f32).astype(e4m3)
        dw = (M - w8.astype(np.float64)).astype(f32).astype(e4m3)
        return w8, dw

    # LayerNorm applied host-side; x-tilde^T pre-transposed and packed as
    # fp8 (r | x8) pairs. The mr column carries the SW weight prescale
    # (its weight-row partner, the colsum row, is left unscaled).
    X = np.asarray(hidden_states, f32).reshape(T, H).astype(np.float64)
    mu = X.mean(axis=1)
    var = X.var(axis=1)
    rstd = 1.0 / np.sqrt(var + EPS)
    xflat = np.zeros((T, HP), np.float64)
    xflat[:, :H] = X * rstd[:, None]
    xflat[:, H] = 1.0
    xflat[:, H + 1] = -mu * rstd * SW
    x8 = xflat.astype(f32).astype(e4m3)
    xr = (xflat - x8.astype(np.float64)).astype(f32).astype(e4m3)
    xpair = np.stack([xr, x8], axis=-1)      # [T, HP, 2]
    # [b, r, t, k, p, s] -> [b*8+r, p, k, s, t]
    xb = np.ascontiguousarray(
        xpair.reshape(2, 8, 128, KT, 128, 2).transpose(0, 1, 4, 3, 5, 2)
        .reshape(16, 128, KT, 2, 128))

    cos2 = np.asarray(cos, f32)[0, 0]       # [S, 64]
    sin2 = np.asarray(sin, f32)[0, 0]
    csn = np.zeros((2, 128, S), bf)
    csn[0] = np.tile(cos2.T, (2, 1)).astype(bf)
    csn[1] = np.tile(sin2.T, (2, 1)).astype(bf)

    # transposed causal mask for scoresT[sk, sq]: keep sk <= sq
    dmask = np.where(np.arange(128)[:, None] <= np.arange(128)[None, :],
                     0.0, NEG).astype(f32)

    NHP = 80
    wq_pad = np.zeros((NHP * HD, H), f32)
    wq_pad[:NH * HD] = np.asarray(wq, f32)
    wdT_pad = np.zeros((NHP * HD, H), f32)
    wdT_pad[:NH * HD] = np.asarray(w_dense, f32).T
    w14 = np.asarray(w_h4h, f32)
    w41T = np.asarray(w_4hh, f32).T         # [F4, H]

    wk_p = pack(np.asarray(wk, f32), prescale=SW)        # [HP, 64]
    wv_p = pack(np.asarray(wv, f32), prescale=SW)

    in_maps = []
    for c in range(8):
        fs = slice(c * F4C_REAL, (c + 1) * F4C_REAL)
        # --- projection weights (fp8 (w8|dw) pairs, x64 prescale) ---
        wpk2 = np.zeros((MT, HP, 128), np.float64)  # [m, contraction row, ch]
        for m in range(5):
            hA = c * NHC + m            # lower-half head (partitions 0..63)
            hB = c * NHC + m + 5        # upper-half head
            wpk2[m, :, 0:64] = pack(wq_pad[hA * 64:(hA + 1) * 64],
                                    scale=0.125, prescale=SW)
            wpk2[m, :, 64:128] = pack(wq_pad[hB * 64:(hB + 1) * 64],
                                      scale=0.125, prescale=SW)
        wpk2[5, :, 0:64] = wk_p
        wpk2[5, :, 64:128] = wv_p
        w14c = pack(w14[fs], prescale=SW)           # [HP, 2272]
        for m in range(6, MT):
            lo = (m - 6) * 128
            hi = min(lo + 128, F4C_REAL)
            wpk2[m, :, 0:hi - lo] = w14c[:, lo:hi]
        w8, dw = fp8_pair(wpk2.reshape(MT * HP, 128))
        wpair = np.stack([w8.reshape(MT, HP, 128), dw.reshape(MT, HP, 128)],
                         axis=2)                     # [m, row, s, c]
        # device layout: [MT, 128 row-within-tile(partition), (ko, s, c)]
        wpk_dev = (wpair.reshape(MT, KT, 128, 2, 128)  # [m, ko, p, s, c]
                   .transpose(0, 2, 1, 3, 4)           # [m, p, ko, s, c]
                   .reshape(MT, 128, KT * 2 * 128))

        # --- dense ct-part weights (bf16, x64 exact prescale) ---
        wc_rows = np.zeros((5 * 128, HP), np.float64)
        wc_rows[:QC, :H] = wdT_pad[c * QC:(c + 1) * QC] * SW
        wddc_dev = (wc_rows.reshape(5, 128, HP).transpose(1, 0, 2)
                    .astype(f32).astype(bf))

        # --- dense gt-part weights fp8 (w8|dw), x64 prescale ---
        wg_rows = np.zeros((18 * 128, HP), np.float64)
        wg_rows[:F4C_REAL, :H] = w41T[fs] * SW
        w8g = wg_rows.astype(f32).astype(e4m3)
        dwg = (wg_rows - w8g.astype(np.float64)).astype(f32).astype(e4m3)
        w8g = w8g.reshape(18, 128, HP)
        dwg = dwg.reshape(18, 128, HP)
        wddg_dev = np.zeros((128, 27, 2, HP), e4m3)
        for kp in range(9):
            wddg_dev[:, kp, 0, :] = w8g[2 * kp]
            wddg_dev[:, kp, 1, :] = w8g[2 * kp + 1]
        for k in range(18):
            wddg_dev[:, 9 + k, 0, :] = w8g[k]
            wddg_dev[:, 9 + k, 1, :] = dwg[k]

        in_maps.append({
            "xb": xb,
            "wpk": np.ascontiguousarray(wpk_dev),
            "wddc": np.ascontiguousarray(wddc_dev),
            "wddg": np.ascontiguousarray(wddg_dev),
            "csn": csn, "dmask": dmask,
        })
    return in_maps


def kernel(hidden_states, attention_mask, cos, sin,
           ln_w1, ln_b1, ln_w2, ln_b2,
           wq, wk, wv, w_dense, w_h4h, w_4hh):
    if "nc" not in _CACHE:
        _CACHE["nc"] = _build()
    nc = _CACHE["nc"]
    in_maps = _prep_inputs(hidden_states, cos, sin, ln_w1, ln_b1, ln_w2, ln_b2,
                           wq, wk, wv, w_dense, w_h4h, w_4hh)
    res = run_bass_kernel_spmd(nc, in_maps, core_ids=list(range(8)))
    acc = np.zeros((T, H), np.float64)
    for r in res.results:
        acc += r["out"][:, :H].astype(np.float64)
    outv = (acc.astype(np.float32)
            + np.asarray(hidden_states, np.float32).reshape(T, H))
    return outv.reshape(B, S, H).astype(np.float32)


# revision 6
# speedup vs baseline: 1.0243x; 1.0243x over previous
"""TRN2 Bass kernel v2 for nn_DecoderLayer_70781061038465 (Falcon-7B style
decoder layer: fractured LayerNorm -> parallel MQA attention + MLP -> residual).

Sharding: 8-way tensor parallelism, no collectives. Each core computes a
partial sum of (attn_out + mlp_out) over its head/MLP shard; the host reduces
the 8 partials and adds the residual.

Design (vs the naive structure):
  - LayerNorm folded entirely into host prep: x-tilde = rstd*(x-mu) prescaled,
    ones/mr columns appended, and the whole operand pre-TRANSPOSED host-side so
    the device only streams contiguous feature-major row-blocks (no XBAR).
  - fp8-e4m3 DoubleRow matmuls with 3-term residual correction for the big
    GEMMs: W ~ w8+dw, x ~ x8+r (both packed as interleaved fp8 pairs); one
    DR matmul per k-tile computes w8^T r + dw^T x8, k-pair DR matmuls compute
    the main w8^T x8 — 0.75x the bf16 PE cycles at BETTER-than-bf16 accuracy.
    Weights carry a x64 prescale (undone on eviction) to keep dw normal-range.
  - Attention (bf16) pipelined inside the MLP projection: per head
    [scores 6][proj hb0][scores 6][proj hb1][ctx], so Act-engine exp latency
    and PSUM-bank recycling hide under matmuls. Gelu is deferred to batched
    in-place groups to avoid Exp<->Gelu activation-table thrash, fused with
    the fp8 (gr|g8) pair quantization feeding the down-projection.
  - dense+down after attention: ct part bf16, gt part fp8-DR, one fused PSUM
    accumulation per 512-wide output chunk across all 8 token row-blocks.
  - Batch pipeline: batch 1's x loads stream during batch 0's dense phase;
    weight tiles prefetched across all phase borders; startup runs the first
    two m-tiles at 128-token-chunk granularity so the PE starts ~11us in.
  - q heads packed two-per-partition-line ([128, 5, S]) via host-side head
    interleave (m, m+5); k duplicated to both partition halves by the PE.
"""
import sys
if "/opt/trn_rl_repo" not in sys.path:
    sys.path.insert(0, "/opt/trn_rl_repo")

from contextlib import ExitStack

import numpy as np
import ml_dtypes

import concourse.bass as bass
import concourse.tile as tile
from concourse import bacc, mybir
from concourse.bass_utils import run_bass_kernel_spmd

F32 = mybir.dt.float32
BF16 = mybir.dt.bfloat16
FP8 = mybir.dt.float8e4
DR = mybir.MatmulPerfMode.DoubleRow
AF = mybir.ActivationFunctionType
MUL = mybir.AluOpType.mult
ADD = mybir.AluOpType.add

# problem shapes (hardcoded per contract)
B, S, H, NH, HD = 2, 1024, 4544, 71, 64
T = B * S                 # 2048 tokens
HP = 4608                 # padded hidden (36*128)
HH = HP // 2              # 2304 half-row width
KT = HP // 128            # 36 contraction tiles
NHC = 10                  # heads per core
QC = NHC * HD             # 640 q channels/core
F4 = 4 * H                # 18176
F4C_REAL = F4 // 8        # 2272
F4C = 2304                # padded (18*128)
MT = 24                   # proj m-tiles: 5 q + 1 kv + 18 mlp
DDK = 23                  # dense contraction tiles: 5 ct + 18 gt
KKG = 6                   # dense k-groups of 4
FC = HP // 512            # 9 output f-chunks
EPS = 1e-5
NEG = -30.0
SW = 64.0                 # fp8 weight prescale (undone at psum eviction)

_CACHE = {}


def _et_chunk(skt, sqc):
    # triangular et storage: (skt<4, sqc in {0,1}) -> 0..7 ; (skt>=4, sqc=1) -> 8..11
    return 2 * skt + sqc if skt < 4 else 4 + skt


def _build():
    nc = bacc.Bacc("TRN2", target_bir_lowering=False, debug=False)
    # x-tilde pre-transposed host-side, fp8 (r|x8) pairs:
    #   [row-block, 128 features, ko, slot(0=r,1=x8), 128 tok]
    xb_d = nc.dram_tensor("xb", [16, 128, KT, 2, 128], FP8, kind="ExternalInput")
    # proj weights fp8 (w8|dw) pairs: [m, 128 row-in-tile, ko, slot, out-ch]
    wpk_d = nc.dram_tensor("wpk", [MT, 128, KT * 2 * 128], FP8,
                           kind="ExternalInput")
    # dense ct-part weights (bf16, x64-prescaled): [128 row-in-tile, 5, HP]
    wddc_d = nc.dram_tensor("wddc", [128, 5, HP], BF16, kind="ExternalInput")
    # dense gt-part weights fp8: rows 0..8 main (w8_2k|w8_2k+1),
    # rows 9..26 corr (w8_k|dw_k); [128, 27, 2, HP]
    wddg_d = nc.dram_tensor("wddg", [128, 27, 2, HP], FP8, kind="ExternalInput")
    cs_d = nc.dram_tensor("csn", [2, 128, S], BF16, kind="ExternalInput")
    dm_d = nc.dram_tensor("dmask", [128, 128], F32, kind="ExternalInput")
    out_d = nc.dram_tensor("out", [T, HP], F32, kind="ExternalOutput")

    xb = xb_d.ap()
    wpk = wpk_d.ap()
    wddc = wddc_d.ap()
    wddg = wddg_d.ap()
    out = out_d.ap()

    with tile.TileContext(nc) as tc, ExitStack() as ctx:
        def pool(name, bufs, space="SBUF"):
            return ctx.enter_context(tc.tile_pool(name=name, bufs=bufs, space=space))

        const = pool("const", 1)
        wpool = pool("wp", 2)
        gts = pool("gts", 3)      # pre-gelu bf16 staging
        res = pool("res", 1)      # per-batch residents: qt/kt2/vt/gt/ct + xt
        et_p = pool("et", 1)
        c2_p = pool("c2", 1)
        wdp = pool("wdp", 2)
        wcp = pool("wcp", 2)
        outp = pool("outp", 3)
        small = pool("small", 4)
        psp = pool("psp", 8, space="PSUM")

        cos_sb = const.tile([128, S], BF16, tag="cos")
        nc.sync.dma_start(cos_sb[:], cs_d.ap()[0])
        sin_sb = const.tile([128, S], BF16, tag="sin")
        nc.sync.dma_start(sin_sb[:], cs_d.ap()[1])
        dmaskT = const.tile([128, 128], F32, tag="dmaskT")
        nc.sync.dma_start(dmaskT[:], dm_d.ap())

        state = {}
        wt_cache = {}
        wdt_cache = {}

        def alloc_batch(b):
            state["qt"] = res.tile([128, 5, S], BF16, tag="qt", name=f"qt{b}")
            state["kt2"] = res.tile([128, S], BF16, tag="kt2", name=f"kt2{b}")
            state["vt"] = res.tile([128, 8, 72], BF16, tag="vt", name=f"vt{b}")
            # fp8 (gr|g8) pairs of the gelu output
            state["gt8"] = res.tile([128, 18, 2, S], FP8, tag="gt8",
                                    name=f"gt8{b}")
            # [p, sqt, head-pair, t]: mid-dim strides keep the transpose AP 3D
            state["ct"] = res.tile([128, 8, 5, 128], BF16, tag="ct", name=f"ct{b}")
            nc.vector.memset(state["vt"][:, :, 64:65], 1.0)

        def alloc_xt(b):
            # [p features, row-block, ko, slot(r|x8), tok]: row-block-major so
            # each host-transposed block loads as one contiguous 9216B DMA
            state["xt"] = res.tile([128, 8, KT, 2, 128], FP8, tag="xt",
                                   name=f"xt{b}")

        def load_wt(b, m):
            wt = wpool.tile([128, KT, 2, 128], FP8, tag="wt", name=f"wt{b}_{m}")
            nc.sync.dma_start(
                wt[:], wpk[m].rearrange("p (ko s c) -> p ko s c", s=2, c=128))
            wt_cache[(b, m)] = wt

        def load_wdt(b, fc, part):
            # part 0/1/2 -> wddg row-groups [0:9], [9:18], [18:27]
            fcols = slice(fc * 512, (fc + 1) * 512)
            wdt = wdp.tile([128, 9, 2, 512], FP8, tag="wdt",
                           name=f"wdt{b}_{fc}_{part}")
            nc.sync.dma_start(wdt[:], wddg[:, 9 * part:9 * part + 9, :, fcols])
            wdt_cache[(b, fc, part)] = wdt

        def load_wdc(b, fc):
            fcols = slice(fc * 512, (fc + 1) * 512)
            wdc = wcp.tile([128, 5, 512], BF16, tag="wdc", name=f"wdc{b}_{fc}")
            nc.sync.dma_start(wdc[:], wddc[:, :, fcols])
            wdt_cache[(b, fc, "c")] = wdc

        def phase_a(b, r):
            """Load one host-transposed, host-LN-prescaled fp8 row-block of
            (r|x8)-packed x-tilde^T straight into xt."""
            nc.sync.dma_start(state["xt"][:, r, :, :, :], xb[b * 8 + r])

        def proj_evict(b, m, hb, ps, wt):
            # psum carries the x64 weight prescale; undo it on eviction
            hcols = slice(hb * 512, hb * 512 + 512)
            if m < 5:
                nc.vector.tensor_scalar_mul(state["qt"][:, m, hcols], ps[:],
                                            1.0 / SW)
            elif m == 5:
                nc.vector.tensor_scalar_mul(state["kt2"][0:64, hcols],
                                            ps[0:64, :], 1.0 / SW)
                nc.vector.tensor_scalar_mul(state["kt2"][64:128, hcols],
                                            ps[0:64, :], 1.0 / SW)
                for j in range(4):
                    r2 = hb * 4 + j
                    pv = psp.tile([128, 72], F32, tag="ps", name=f"pv{b}_{r2}")
                    xr = state["xt"]
                    for kp in range(KT // 2):
                        nc.tensor.matmul(
                            pv[:, :64],
                            xr[:, r2, 2 * kp:2 * kp + 2, 1, :],
                            wt[:, 2 * kp:2 * kp + 2, 0, 64:128],
                            start=(kp == 0), stop=False, perf_mode=DR)
                    for k in range(KT):
                        nc.tensor.matmul(
                            pv[:, :64],
                            xr[:, r2, k, :, :],
                            wt[:, k, :, 64:128],
                            start=False, stop=(k == KT - 1), perf_mode=DR)
                    nc.vector.tensor_scalar_mul(state["vt"][:, r2, :64],
                                                pv[:, :64], 1.0 / SW)
            else:
                # raw (pre-gelu) bf16 evict to scratch; gelu + fp8 pair
                # quantization happen later in batches
                tt = m - 6
                if hb == 0:
                    state[f"scr{tt}"] = gts.tile([128, S], BF16, tag="scr",
                                                 name=f"scr{b}_{tt}")
                nc.vector.tensor_scalar_mul(state[f"scr{tt}"][:, hcols],
                                            ps[:], 1.0 / SW)

        def proj_hb(b, m, hb):
            xt = state["xt"]
            if (b, m) not in wt_cache:
                load_wt(b, m)
            wt = wt_cache[(b, m)]
            hcols = slice(hb * 512, hb * 512 + 512)
            ps = psp.tile([128, 512], F32, tag="ps", name=f"ps{b}_{m}_{hb}")
            rb = slice(hb * 4, (hb + 1) * 4)
            for kp in range(KT // 2):
                nc.tensor.matmul(
                    ps[:], wt[:, 2 * kp:2 * kp + 2, 0, :],
                    xt[:, rb, 2 * kp:2 * kp + 2, 1, :].rearrange(
                        "p r k t -> p k r t"),
                    start=(kp == 0), stop=False, perf_mode=DR)
            for k in range(KT):
                nc.tensor.matmul(
                    ps[:], wt[:, k, :, :],
                    xt[:, rb, k, :, :].rearrange("p r j t -> p j r t"),
                    start=False, stop=(k == KT - 1), perf_mode=DR)
            proj_evict(b, m, hb, ps, wt)
            if hb == 1:
                wt_cache.pop((b, m))

        def proj_m(b, m):
            proj_hb(b, m, 0)
            proj_hb(b, m, 1)

        chunk_ps = {}

        def proj_chunk(b, m, c):
            """128-token-chunk projection for startup: chunk c becomes ready
            as soon as row-block c is transposed."""
            xt = state["xt"]
            wt = wt_cache[(b, m)]
            hb = c // 4
            key = (b, m, hb)
            if key not in chunk_ps:
                chunk_ps[key] = psp.tile([128, 512], F32, tag="ps",
                                         name=f"ps{b}_{m}_{hb}")
            ps = chunk_ps[key]
            col = (c % 4) * 128
            for kp in range(KT // 2):
                nc.tensor.matmul(
                    ps[:, col:col + 128], wt[:, 2 * kp:2 * kp + 2, 0, :],
                    xt[:, c, 2 * kp:2 * kp + 2, 1, :],
                    start=(kp == 0), stop=False, perf_mode=DR)
            for k in range(KT):
                nc.tensor.matmul(
                    ps[:, col:col + 128], wt[:, k, :, :],
                    xt[:, c, k, :, :],
                    start=False, stop=(k == KT - 1), perf_mode=DR)
            if c % 4 == 3:
                proj_evict(b, m, hb, chunk_ps.pop(key), wt)
                if hb == 1:
                    wt_cache.pop((b, m))

        def gelu_batch(tiles):
            gt8 = state["gt8"]
            for tt in tiles:
                scr = state.pop(f"scr{tt}")
                nc.scalar.activation(scr[:], scr[:], AF.Gelu)
                nc.vector.tensor_copy(gt8[:, tt, 1, :], scr[:])
                nc.vector.tensor_tensor(gt8[:, tt, 0, :], scr[:],
                                        gt8[:, tt, 1, :],
                                        op=mybir.AluOpType.subtract)

        def rope_tile(t):
            # t: [128, S] bf16; rotate-half on both 64-row halves
            rot = c2_p.tile([128, 8, 128], BF16, tag="c2", name="rot")
            rot = rot[:].rearrange("p a b -> p (a b)")
            nc.vector.tensor_scalar_mul(rot[0:32, :], t[32:64, :], -1.0)
            nc.vector.tensor_copy(rot[32:64, :], t[0:32, :])
            nc.vector.tensor_scalar_mul(rot[64:96, :], t[96:128, :], -1.0)
            nc.vector.tensor_copy(rot[96:128, :], t[64:96, :])
            nc.vector.tensor_mul(t, t, cos_sb[:])
            nc.vector.tensor_mul(rot[:], rot[:], sin_sb[:])
            nc.vector.tensor_add(t, t, rot[:])

        def rope_all(b):
            for slot in range(5):
                rope_tile(state["qt"][:, slot, :])
            rope_tile(state["kt2"][:])

        def attn_scores(b, h, part):
            half, slot = h // 5, h % 5
            base = 64 * half
            qt, kt2 = state["qt"], state["kt2"]
            if part == 0:
                et = et_p.tile([128, 12, 512], BF16, tag="et",
                               name=f"et{b}_{h}")
                state["et"] = et
            et = state["et"]
            skts = range(0, 3) if part == 0 else range(3, 8)
            for skt in skts:
                for sqc in range(skt // 4, 2):
                    sp = psp.tile([128, 512], F32, tag="ps",
                                  name=f"sp{b}_{h}_{skt}_{sqc}")
                    nc.tensor.matmul(
                        sp[:], kt2[base:base + 64, skt * 128:(skt + 1) * 128],
                        qt[base:base + 64, slot, sqc * 512:(sqc + 1) * 512],
                        start=True, stop=True)
                    if skt // 4 == sqc:
                        lc = skt * 128 - sqc * 512
                        nc.vector.tensor_tensor(
                            sp[:, lc:lc + 128], sp[:, lc:lc + 128],
                            dmaskT[:], op=ADD)
                    nc.scalar.activation(
                        et[:, _et_chunk(skt, sqc), :], sp[:], AF.Exp)

        def attn_ctx(b, h):
            vt, ct, et = state["vt"], state["ct"], state["et"]
            if h % 2 == 0:
                state["c2"] = c2_p.tile([128, 8, 128], BF16, tag="c2",
                                        name=f"c2{b}_{h}")
            c2 = state["c2"]
            for sqt in range(8):
                cp = psp.tile([128, 72], F32, tag="ps", name=f"cp{b}_{h}_{sqt}")
                sqc = sqt // 4
                off = sqt * 128 - sqc * 512
                for skt in range(sqt + 1):
                    nc.tensor.matmul(
                        cp[:, :65],
                        et[:, _et_chunk(skt, sqc), off:off + 128],
                        vt[:, skt, :65],
                        start=(skt == 0), stop=(skt == sqt))
                recd = small.tile([128, 1], F32, tag="recd")
                nc.vector.reciprocal(recd[:], cp[:, 64:65])
                nc.vector.tensor_scalar_mul(
                    c2[:, sqt, (h % 2) * 64:(h % 2) * 64 + 64],
                    cp[:, :64], recd[:])
            if h % 2 == 1:
                nc.sync.dma_start_transpose(
                    ct[:, :, h // 2, :],
                    c2[:].rearrange("p a b -> p (a b)"))

        def dense_fc(b, fc, extra=None):
            gt8, ct = state["gt8"], state["ct"]
            fcols = slice(fc * 512, (fc + 1) * 512)
            pss = [psp.tile([128, 512], F32, tag="ps",
                            name=f"pd{b}_{fc}_{i}") for i in range(8)]
            # ct part, bf16
            if (b, fc, "c") not in wdt_cache:
                load_wdc(b, fc)
            wdc = wdt_cache.pop((b, fc, "c"))
            if (b, fc, 0) not in wdt_cache:
                load_wdt(b, fc, 0)
            for kk in range(5):
                for r in range(8):
                    nc.tensor.matmul(pss[r][:], ct[:, r, kk, :],
                                     wdc[:, kk, :],
                                     start=(kk == 0), stop=False)
            # gt main: g8 k-pairs x w8 k-pairs
            wdt = wdt_cache.pop((b, fc, 0))
            load_wdt(b, fc, 1)
            for kp in range(9):
                for r in range(8):
                    rcols = slice(r * 128, (r + 1) * 128)
                    nc.tensor.matmul(
                        pss[r][:],
                        gt8[:, 2 * kp:2 * kp + 2, 1, rcols],
                        wdt[:, kp, :, :],
                        start=False, stop=False, perf_mode=DR)
            # gt corrections: (gr|g8) x (w8|dw)
            for part in (1, 2):
                wdt = wdt_cache.pop((b, fc, part))
                if part == 1:
                    load_wdt(b, fc, 2)
                    if fc + 1 < FC:
                        load_wdc(b, fc + 1)
                    if extra is not None:
                        extra()
                for kx in range(9):
                    k = (part - 1) * 9 + kx
                    for r in range(8):
                        rcols = slice(r * 128, (r + 1) * 128)
                        nc.tensor.matmul(
                            pss[r][:],
                            gt8[:, k, :, rcols],
                            wdt[:, kx, :, :],
                            start=False, stop=(k == 17), perf_mode=DR)
            for r in range(8):
                osb = outp.tile([128, 512], F32, tag="osb")
                nc.vector.tensor_scalar_mul(osb[:], pss[r][:], 1.0 / SW)
                nc.sync.dma_start(
                    out[b * S + r * 128: b * S + (r + 1) * 128, fcols], osb[:])

        # batched gelu groups: BEFORE unit h, which gt tiles to activate
        # (keeps at most 3 pre-gelu scratch tiles alive)
        GELU_SCHED = {1: range(0, 3), 3: range(3, 5), 5: range(5, 7),
                      7: range(7, 9), 9: range(9, 11)}

        def batch_body(b, startup=False, pipelined_next=False):
            alloc_batch(b)
            if startup:
                # chunked m0/m1: each 128-token chunk starts right after its
                # row-block transpose lands
                load_wt(b, 0)
                phase_a(b, 0)
                load_wt(b, 1)
                for r in range(8):
                    if r < 7:
                        phase_a(b, r + 1)
                    proj_chunk(b, 0, r)
                    proj_chunk(b, 1, r)
                for m in range(2, 6):
                    proj_m(b, m)
            else:
                for m in range(6):
                    proj_m(b, m)
            rope_all(b)
            proj_m(b, 6)
            proj_m(b, 7)
            for h in range(10):
                if h in GELU_SCHED:
                    gelu_batch(GELU_SCHED[h])
                attn_scores(b, h, 0)
                proj_hb(b, 8 + h, 0)
                attn_scores(b, h, 1)
                proj_hb(b, 8 + h, 1)
                attn_ctx(b, h)
            gelu_batch(range(11, 12))
            load_wdc(b, 0)
            load_wdt(b, 0, 0)
            for m in range(18, 20):
                proj_m(b, m)
            gelu_batch(range(12, 14))
            for m in range(20, 22):
                proj_m(b, m)
            gelu_batch(range(14, 16))
            for m in range(22, 24):
                proj_m(b, m)
            gelu_batch(range(16, 18))
            if pipelined_next:
                alloc_xt(1)

                def step(fc):
                    def run():
                        phase_a(1, fc - 1)
                        if fc == 6:
                            load_wt(1, 0)
                        elif fc == 7:
                            load_wt(1, 1)
                    return run
            for fc in range(FC):
                extra = step(fc) if (pipelined_next and 1 <= fc <= 8) else None
                dense_fc(b, fc, extra=extra)

        alloc_xt(0)
        batch_body(0, startup=True, pipelined_next=True)
        batch_body(1)

    nc.compile()
    return nc


def _prep_inputs(hidden_states, cos, sin, ln_w1, ln_b1, ln_w2, ln_b2,
                 wq, wk, wv, w_dense, w_h4h, w_4hh):
    f32 = np.float32
    bf = ml_dtypes.bfloat16
    e4m3 = ml_dtypes.float8_e4m3
    lnw = np.concatenate([np.asarray(ln_w1), np.asarray(ln_w2)]).astype(np.float64)
    lnb = np.concatenate([np.asarray(ln_b1), np.asarray(ln_b2)]).astype(np.float64)

    def pack(Wc, scale=1.0, prescale=1.0):
        # Wc [O, H] -> [HP, O] f64: ln-folded + bias row + colsum row + pad.
        # prescale multiplies all rows EXCEPT the colsum row (its x-side
        # partner, the mr column, carries the prescale instead).
        W64 = Wc.astype(np.float64) * scale
        Wp = W64 * lnw                      # [O, H]
        bias = W64 @ lnb                    # [O]
        cw = Wp.sum(axis=1)                 # [O]
        O = Wc.shape[0]
        outw = np.zeros((HP, O), np.float64)
        outw[:H] = Wp.T * prescale
        outw[H] = bias * prescale
        outw[H + 1] = cw
        return outw

    def fp8_pair(M):
        # M f64 [HP, O] -> (w8, dw) e4m3
        w8 = M.astype(f32).astype(e4m3)
        dw = (M - w8.astype(np.float64)).astype(f32).astype(e4m3)
        return w8, dw

    # LayerNorm applied host-side; x-tilde^T pre-transposed and packed as
    # fp8 (r | x8) pairs. The mr column carries the SW weight prescale
    # (its weight-row partner, the colsum row, is left unscaled).
    X = np.asarray(hidden_states, f32).reshape(T, H).astype(np.float64)
    mu = X.mean(axis=1)
    var = X.var(axis=1)
    rstd = 1.0 / np.sqrt(var + EPS)
    xflat = np.zeros((T, HP), np.float64)
    xflat[:, :H] = X * rstd[:, None]
    xflat[:, H] = 1.0
    xflat[:, H + 1] = -mu * rstd * SW
    x8 = xflat.astype(f32).astype(e4m3)
    xr = (xflat - x8.astype(np.float64)).astype(f32).astype(e4m3)
    xpair = np.stack([xr, x8], axis=-1)      # [T, HP, 2]
    # [b, r, t, k, p, s] -> [b*8+r, p, k, s, t]
    xb = np.ascontiguousarray(
        xpair.reshape(2, 8, 128, KT, 128, 2).transpose(0, 1, 4, 3, 5, 2)
        .reshape(16, 128, KT, 2, 128))

    cos2 = np.asarray(cos, f32)[0, 0]       # [S, 64]
    sin2 = np.asarray(sin, f32)[0, 0]
    csn = np.zeros((2, 128, S), bf)
    csn[0] = np.tile(cos2.T, (2, 1)).astype(bf)
    csn[1] = np.tile(sin2.T, (2, 1)).astype(bf)

    # transposed causal mask for scoresT[sk, sq]: keep sk <= sq
    dmask = np.where(np.arange(128)[:, None] <= np.arange(128)[None, :],
                     0.0, NEG).astype(f32)

    NHP = 80
    wq_pad = np.zeros((NHP * HD, H), f32)
    wq_pad[:NH * HD] = np.asarray(wq, f32)
    wdT_pad = np.zeros((NHP * HD, H), f32)
    wdT_pad[:NH * HD] = np.asarray(w_dense, f32).T
    w14 = np.asarray(w_h4h, f32)
    w41T = np.asarray(w_4hh, f32).T         # [F4, H]

    wk_p = pack(np.asarray(wk, f32), prescale=SW)        # [HP, 64]
    wv_p = pack(np.asarray(wv, f32), prescale=SW)

    in_maps = []
    for c in range(8):
        fs = slice(c * F4C_REAL, (c + 1) * F4C_REAL)
        # --- projection weights (fp8 (w8|dw) pairs, x64 prescale) ---
        wpk2 = np.zeros((MT, HP, 128), np.float64)  # [m, contraction row, ch]
        for m in range(5):
            hA = c * NHC + m            # lower-half head (partitions 0..63)
            hB = c * NHC + m + 5        # upper-half head
            wpk2[m, :, 0:64] = pack(wq_pad[hA * 64:(hA + 1) * 64],
                                    scale=0.125, prescale=SW)
            wpk2[m, :, 64:128] = pack(wq_pad[hB * 64:(hB + 1) * 64],
                                      scale=0.125, prescale=SW)
        wpk2[5, :, 0:64] = wk_p
        wpk2[5, :, 64:128] = wv_p
        w14c = pack(w14[fs], prescale=SW)           # [HP, 2272]
        for m in range(6, MT):
            lo = (m - 6) * 128
            hi = min(lo + 128, F4C_REAL)
            wpk2[m, :, 0:hi - lo] = w14c[:, lo:hi]
        w8, dw = fp8_pair(wpk2.reshape(MT * HP, 128))
        wpair = np.stack([w8.reshape(MT, HP, 128), dw.reshape(MT, HP, 128)],
                         axis=2)                     # [m, row, s, c]
        # device layout: [MT, 128 row-within-tile(partition), (ko, s, c)]
        wpk_dev = (wpair.reshape(MT, KT, 128, 2, 128)  # [m, ko, p, s, c]
                   .transpose(0, 2, 1, 3, 4)           # [m, p, ko, s, c]
                   .reshape(MT, 128, KT * 2 * 128))

        # --- dense ct-part weights (bf16, x64 exact prescale) ---
        wc_rows = np.zeros((5 * 128, HP), np.float64)
        wc_rows[:QC, :H] = wdT_pad[c * QC:(c + 1) * QC] * SW
        wddc_dev = (wc_rows.reshape(5, 128, HP).transpose(1, 0, 2)
                    .astype(f32).astype(bf))

        # --- dense gt-part weights fp8 (w8|dw), x64 prescale ---
        wg_rows = np.zeros((18 * 128, HP), np.float64)
        wg_rows[:F4C_REAL, :H] = w41T[fs] * SW
        w8g = wg_rows.astype(f32).astype(e4m3)
        dwg = (wg_rows - w8g.astype(np.float64)).astype(f32).astype(e4m3)
        w8g = w8g.reshape(18, 128, HP)
        dwg = dwg.reshape(18, 128, HP)
        wddg_dev = np.zeros((128, 27, 2, HP), e4m3)
        for kp in range(9):
            wddg_dev[:, kp, 0, :] = w8g[2 * kp]
            wddg_dev[:, kp, 1, :] = w8g[2 * kp + 1]
        for k in range(18):
            wddg_dev[:, 9 + k, 0, :] = w8g[k]
            wddg_dev[:, 9 + k, 1, :] = dwg[k]

        in_maps.append({
            "xb": xb,
            "wpk": np.ascontiguousarray(wpk_dev),
            "wddc": np.ascontiguousarray(wddc_dev),
            "wddg": np.ascontiguousarray(wddg_dev),
            "csn": csn, "dmask": dmask,
        })
    return in_maps


def kernel(hidden_states, attention_mask, cos, sin,
           ln_w1, ln_b1, ln_w2, ln_b2,
           wq, wk, wv, w_dense, w_h4h, w_4hh):
    if "nc" not in _CACHE:
        _CACHE["nc"] = _build()
    nc = _CACHE["nc"]
    in_maps = _prep_inputs(hidden_states, cos, sin, ln_w1, ln_b1, ln_w2, ln_b2,
                           wq, wk, wv, w_dense, w_h4h, w_4hh)
    res = run_bass_kernel_spmd(nc, in_maps, core_ids=list(range(8)))
    acc = np.zeros((T, H), np.float64)
    for r in res.results:
        acc += r["out"][:, :H].astype(np.float64)
    outv = (acc.astype(np.float32)
            + np.asarray(hidden_states, np.float32).reshape(T, H))
    return outv.reshape(B, S, H).astype(np.float32)
